# revision 13
# baseline (speedup 1.0000x reference)
"""Trainium2 Bass kernel for CasimirSparseAttention.

Math (per batch b):
    S = (x_b @ x_b.T) / sqrt(D)                      # (T, T)
    probs = softmax(S, axis=-1)
    kept = probs >= 0.01  (vacuum = probs < 0.01)
    vac_sum = sum(probs * ~kept)
    casimir[t, o] = vac_sum[t] * rowsum_W[o]          # vac_in is const across D
    attended = (probs*kept) @ x_b / (sum(probs*kept) + 1e-9)
    out = attended + 0.01 * casimir

Working in unnormalized exp-space (E = exp(S/sqrt(D)), row sum = sa):
    kept mask:  E >= 0.01 * sa
    attended = (E*mask) @ x_b / (sum(E*mask) + 1e-9*sa)
    beta     = 0.01 * (1 - sum(E*mask)/sa);  out += beta * rowsum_W

Sharding: 8 cores = (batch b in 0..3) x (half of T). Each core computes
1024 query rows against all 2048 keys of its batch.

Per-core pipeline over 8 row-blocks of 128 queries (software-pipelined:
S for block i+1 is issued on PE before the transposes/attended of block
i, so PE never waits on the exp->mask chain):
    PE   : S-block via fp8 DoubleRow matmuls (xq^T stationary, x^T moving)
    ACT  : exp(scale*S) PSUM->SBUF directly in bf16, free-dim accum -> sa
    DVE  : threshold mask + masked E, all bf16 (2x DVE rate), one reduce
           per half for the kept sum (quantized sum: its rounding error
           cancels against the identically-quantized matmul weights)
    PE   : transpose masked-E chunks (128x128), then attended matmul
           in single-pass bf16 (x as bf16; ~0.2%/elem, rel-err gate 2e-2)
    DVE  : transposed-chunk PSUM->SBUF copies (off ACT so they don't
           queue behind the next block's exp)
    ACT  : scale attended rows by 1/(kept + 1e-9*sa)
    DVE  : add beta * rowsum_W rank-1 term

DMA trigger instructions serialize at ~0.6us each on the issuing queue,
so inputs are staged as FEW large transfers (host pre-lays them out so
each is a contiguous-per-partition 2D pattern), ordered so the transfers
gating the first S matmuls come first.
"""

import sys

sys.path.insert(0, "/opt/trn_rl_repo")

from contextlib import ExitStack

import numpy as np

from concourse import bacc, mybir, tile
from concourse.bass_utils import run_bass_kernel_spmd

F32 = mybir.dt.float32
BF16 = mybir.dt.bfloat16
OP = mybir.AluOpType
AFT = mybir.ActivationFunctionType

P = 128          # partitions / row-block size
T = 2048         # keys per batch
D = 1024         # model dim
QR = 1024        # query rows per core
NBLK = QR // P   # 8 row blocks per core
NT_CHUNKS = T // P    # 16 t-chunks
NC = D // 256    # 4 contraction chunks (fp8 DoubleRow: K=256 each)
NG = T // 512    # 4 column groups for S
NH = 8           # xh transfer groups (2 t-chunks each)
SCALE = float(1.0 / np.sqrt(np.float32(D)))   # 0.03125
THRESH = 0.01
EPS = 1e-9

_CACHE = {}


def _build():
    nc = bacc.Bacc("TRN2", target_bir_lowering=False, debug=False)

    FP8 = mybir.dt.float8e4
    # fp8 DoubleRow operands; layouts put the partition dim second so each
    # [index] slice is one contiguous-per-partition 2D DMA:
    #   xt8[g] = x_b.T, all 4 K-chunks, t-cols 512g..512(g+1)
    #   xq8[i] = x_b.T, all 4 K-chunks, q-cols 128i..128(i+1)
    xt8 = nc.dram_tensor("xt8", [NG, P, NC, 2, 512], FP8, kind="ExternalInput")
    xq8 = nc.dram_tensor("xq8", [NBLK, P, NC, 2, P], FP8, kind="ExternalInput")
    xh = nc.dram_tensor("xh", [NH, P, 2, D], BF16, kind="ExternalInput")
    wb = nc.dram_tensor("wb", [P, D], F32, kind="ExternalInput")       # rowsum_W bcast
    ident = nc.dram_tensor("ident", [P, P], BF16, kind="ExternalInput")
    out = nc.dram_tensor("out", [QR, D], F32, kind="ExternalOutput")

    out_ap = out.ap()

    with tile.TileContext(nc) as tc, ExitStack() as ctx:
        # resident operands
        p_xt = ctx.enter_context(tc.tile_pool(name="xt", bufs=NG))
        p_xq = ctx.enter_context(tc.tile_pool(name="xq", bufs=NBLK))
        p_xn = ctx.enter_context(tc.tile_pool(name="xn", bufs=NH))
        p_cst = ctx.enter_context(tc.tile_pool(name="cst", bufs=3))
        # per-block working tiles
        p_exp = ctx.enter_context(tc.tile_pool(name="exp", bufs=5))
        p_msk = ctx.enter_context(tc.tile_pool(name="msk", bufs=5))
        p_pk = ctx.enter_context(tc.tile_pool(name="pk", bufs=6))
        p_pkt = ctx.enter_context(tc.tile_pool(name="pkt", bufs=7))
        p_out = ctx.enter_context(tc.tile_pool(name="o", bufs=2))
        p_wt = ctx.enter_context(tc.tile_pool(name="wt", bufs=4))
        p_sm = ctx.enter_context(tc.tile_pool(name="sm", bufs=40))
        # PSUM
        p_ps_s = ctx.enter_context(tc.tile_pool(name="ps_s", bufs=2, space="PSUM"))
        p_ps_a = ctx.enter_context(tc.tile_pool(name="ps_a", bufs=2, space="PSUM"))
        p_ps_t = ctx.enter_context(tc.tile_pool(name="ps_t", bufs=2, space="PSUM"))

        # PE warmup: the tensor engine clocks up only after ~3us of
        # continuous activity (cold matmuls run ~2.7x slower), so chew
        # on dummy matmuls while the input DMAs fill
        warm = p_cst.tile([P, P], BF16, tag="warm")
        nc.gpsimd.memset(warm[:], 0.0)
        wps = p_ps_s.tile([P, P], F32, tag="s", name="warmup")
        for _ in range(40):
            nc.tensor.matmul(wps[:], lhsT=warm[:], rhs=warm[:],
                             start=True, stop=True)

        # transfers in criticality order; all on the sync queue so earlier
        # ones get the DMA bandwidth first
        xq_sb = [None] * NBLK
        xt_sb = [None] * NG

        def load_xq(i):
            tq = p_xq.tile([P, NC, 2, P], FP8, tag="xq", name="tq")
            nc.sync.dma_start(tq[:], xq8.ap()[i])
            xq_sb[i] = tq

        load_xq(0)
        for g in range(NG):
            tt = p_xt.tile([P, NC, 2, 512], FP8, tag="xt", name="tt")
            nc.sync.dma_start(tt[:], xt8.ap()[g])
            xt_sb[g] = tt
        load_xq(1)
        id_sb = p_cst.tile([P, P], BF16, tag="id")
        nc.sync.dma_start(id_sb[:], ident.ap())
        xh_sb = []
        for jg in range(NH):
            t_ = p_xn.tile([P, 2, D], BF16, tag="xn")
            nc.sync.dma_start(t_[:], xh.ap()[jg])
            xh_sb.append(t_)
            if jg % 2 == 1 and jg // 2 + 2 < NBLK:
                load_xq(jg // 2 + 2)
        for i in range(NH // 2 + 2, NBLK):
            load_xq(i)
        wb_sb = p_cst.tile([P, D], F32, tag="wb")
        nc.sync.dma_start(wb_sb[:], wb.ap())

        def s_block(i):
            """S matmuls + exp for row-block i; returns per-half E and sa."""
            exp_halves, sum_parts = [], []
            for half in range(2):
                sp = p_ps_s.tile([P, T // 2], F32, tag="s", name="sp")
                for k in range(2):
                    g = half * 2 + k
                    for c in range(NC):
                        nc.tensor.matmul(
                            sp[:, k * 512:(k + 1) * 512],
                            lhsT=xq_sb[i][:, c],
                            rhs=xt_sb[g][:, c],
                            start=(c == 0), stop=(c == NC - 1),
                            perf_mode=mybir.MatmulPerfMode.DoubleRow)
                ex = p_exp.tile([P, T // 2], BF16, tag="ex")
                sa = p_sm.tile([P, 1], F32, tag="sm")
                nc.scalar.activation(ex[:], sp[:], AFT.Exp, scale=SCALE,
                                     accum_out=sa[:])
                exp_halves.append(ex)
                sum_parts.append(sa)
            return exp_halves, sum_parts

        def rest_block(i, exp_halves, sum_parts):
            """Mask, transpose, attended matmul, epilogue for row-block i."""
            sum_all = p_sm.tile([P, 1], F32, tag="sm")
            nc.gpsimd.tensor_tensor(sum_all[:], sum_parts[0][:],
                                    sum_parts[1][:], OP.add)
            thr = p_sm.tile([P, 1], F32, tag="sm")
            nc.gpsimd.tensor_scalar(out=thr[:], in0=sum_all[:],
                                    scalar1=THRESH, scalar2=None, op0=OP.mult)

            pk_halves, keptq_parts = [], []
            for half in range(2):
                mk = p_msk.tile([P, T // 2], BF16, tag="mk")
                nc.vector.tensor_scalar(out=mk[:], in0=exp_halves[half][:],
                                        scalar1=thr[:], scalar2=None,
                                        op0=OP.is_ge)
                pk = p_pk.tile([P, T // 2], BF16, tag="pk")
                nc.vector.tensor_tensor(pk[:], exp_halves[half][:], mk[:],
                                        OP.mult)
                pk_halves.append(pk)

            att = [p_ps_a.tile([P, 512], F32, tag="a", name="att")
                   for _ in range(2)]
            # transpose groups into PSUM, one wide copy out per group;
            # first group is a singleton so attended starts sooner
            groups = [[0], [1, 2, 3]] + [
                list(range(g, g + 4)) for g in range(4, NT_CHUNKS, 4)]
            for grp in groups:
                tp = p_ps_t.tile([P, len(grp) * P], BF16, tag="t", name="tp")
                for jj, j in enumerate(grp):
                    src = pk_halves[j // 8][:, (j % 8) * P:(j % 8 + 1) * P]
                    nc.tensor.transpose(tp[:, jj * P:(jj + 1) * P], src,
                                        id_sb[:])
                pkt = p_pkt.tile([P, len(grp) * P], BF16, tag="pkt",
                                 name="pkt")
                nc.vector.tensor_copy(pkt[:], tp[:])
                for jj, j in enumerate(grp):
                    lhs = pkt[:, jj * P:(jj + 1) * P]
                    for k in range(2):
                        dcols = slice(k * 512, (k + 1) * 512)
                        nc.tensor.matmul(att[k][:], lhsT=lhs,
                                         rhs=xh_sb[j // 2][:, j % 2, dcols],
                                         start=(j == 0),
                                         stop=(j == NT_CHUNKS - 1))
            return att, pk_halves, sum_all

        def tail_block(i, att, pk_halves, sum_all):
            """Kept sums, alpha/beta, epilogue for row-block i. Emitted
            after part1 of block i+1 so the next block's mask/mult run
            ahead of these on DVE."""
            keptq_parts = []
            # the quantized (bf16) kept sum serves both alpha and beta
            for half in range(2):
                kq = p_sm.tile([P, 1], F32, tag="sm", name="kq")
                nc.vector.tensor_reduce(kq[:], pk_halves[half][:],
                                        mybir.AxisListType.X, OP.add)
                keptq_parts.append(kq)
            kept_q = p_sm.tile([P, 1], F32, tag="sm")
            nc.gpsimd.tensor_tensor(kept_q[:], keptq_parts[0][:],
                                    keptq_parts[1][:], OP.add)

            # alpha = 1 / (kept_q + eps * sum_all)
            den = p_sm.tile([P, 1], F32, tag="sm")
            nc.gpsimd.tensor_scalar(out=den[:], in0=sum_all[:], scalar1=EPS,
                                    scalar2=None, op0=OP.mult)
            nc.gpsimd.tensor_tensor(den[:], den[:], kept_q[:], OP.add)
            alpha = p_sm.tile([P, 1], F32, tag="sm")
            nc.vector.reciprocal(alpha[:], den[:])
            # beta = 0.01 * (1 - kept_q / sum_all)
            rsum = p_sm.tile([P, 1], F32, tag="sm")
            nc.vector.reciprocal(rsum[:], sum_all[:])
            beta = p_sm.tile([P, 1], F32, tag="sm")
            nc.gpsimd.tensor_tensor(beta[:], kept_q[:], rsum[:], OP.mult)
            nc.gpsimd.tensor_scalar(out=beta[:], in0=beta[:], scalar1=-THRESH,
                                    scalar2=THRESH, op0=OP.mult, op1=OP.add)

            # epilogue per d-half so the output DMA overlaps the other half;
            # wt/add on gpsimd so DVE stays free for the next block's
            # mask/mult/copies
            o_sb = p_out.tile([P, D], F32, tag="o")
            for k in range(2):
                dcols = slice(k * 512, (k + 1) * 512)
                nc.scalar.mul(o_sb[:, dcols], att[k][:], alpha[:])
                wt = p_wt.tile([P, 512], F32, tag="wt")
                nc.scalar.mul(wt[:], wb_sb[:, dcols], beta[:])
                nc.gpsimd.tensor_tensor(o_sb[:, dcols], o_sb[:, dcols],
                                        wt[:], OP.add)
                nc.sync.dma_start(out_ap[i * P:(i + 1) * P, dcols],
                                  o_sb[:, dcols])

        # software pipeline: S for block i+1 runs on PE while the
        # exp/mask/mult chain of block i fills; block i's tail (kept sums,
        # epilogue) is emitted after block i+1's mask/mult so those jump
        # ahead in the DVE queue
        pending = s_block(0)
        prev = None
        for i in range(NBLK):
            nxt = s_block(i + 1) if i + 1 < NBLK else None
            cur = rest_block(i, *pending)
            if prev is not None:
                tail_block(i - 1, *prev)
            prev = cur
            pending = nxt
        tail_block(NBLK - 1, *prev)

    nc.compile()
    return nc


def get_nc():
    if "nc" not in _CACHE:
        _CACHE["nc"] = _build()
    return _CACHE["nc"]


def make_in_maps(x, W):
    import ml_dtypes
    bf = ml_dtypes.bfloat16
    f8 = ml_dtypes.float8_e4m3
    x = np.asarray(x, dtype=np.float32)
    W = np.asarray(W, dtype=np.float32)
    wrow = W.sum(axis=1, dtype=np.float32)                      # (D,)
    wb = np.ascontiguousarray(np.broadcast_to(wrow, (P, D))).astype(np.float32)
    ident = np.eye(P, dtype=bf)
    in_maps = []
    for core in range(8):
        b, h = core // 2, core % 2
        xb = x[b]                                               # (T, D)
        xt_f8 = np.ascontiguousarray(xb.T).astype(f8)           # (D, T)
        xt_dr = xt_f8.reshape(NC, P, 2, T)                      # DoubleRow pairs
        # [g, P, NC, 2, 512]: partition-major contiguous per transfer
        xt8 = np.ascontiguousarray(
            xt_dr.reshape(NC, P, 2, NG, 512).transpose(3, 1, 0, 2, 4))
        xq_dr = xt_dr[:, :, :, h * QR:(h + 1) * QR]             # (NC,P,2,QR)
        xq8 = np.ascontiguousarray(
            xq_dr.reshape(NC, P, 2, NBLK, P).transpose(3, 1, 0, 2, 4))
        xh_bf = np.ascontiguousarray(
            xb.astype(bf).reshape(NH, 2, P, D).transpose(0, 2, 1, 3))
        in_maps.append({"xt8": xt8, "xq8": xq8, "xh": xh_bf,
                        "wb": wb, "ident": ident})
    return in_maps


def kernel(x, W):
    nc = get_nc()
    in_maps = make_in_maps(x, W)
    res = run_bass_kernel_spmd(nc, in_maps, list(range(8)))
    out = np.empty((4, T, D), dtype=np.float32)
    for core in range(8):
        b, h = core // 2, core % 2
        out[b, h * QR:(h + 1) * QR, :] = res.results[core]["out"]
    return out


# revision 17
# speedup vs baseline: 1.0190x; 1.0190x over previous
"""Trainium2 Bass kernel for CasimirSparseAttention.

Math (per batch b):
    S = (x_b @ x_b.T) / sqrt(D)                      # (T, T)
    probs = softmax(S, axis=-1)
    kept = probs >= 0.01  (vacuum = probs < 0.01)
    vac_sum = sum(probs * ~kept)
    casimir[t, o] = vac_sum[t] * rowsum_W[o]          # vac_in is const across D
    attended = (probs*kept) @ x_b / (sum(probs*kept) + 1e-9)
    out = attended + 0.01 * casimir

Working in unnormalized exp-space (E = exp(S/sqrt(D)), row sum = sa):
    kept mask:  E >= 0.01 * sa
    attended = (E*mask) @ x_b / (sum(E*mask) + 1e-9*sa)
    beta     = 0.01 * (1 - sum(E*mask)/sa);  out += beta * rowsum_W

Sharding: 8 cores = (batch b in 0..3) x (half of T). Each core computes
1024 query rows against all 2048 keys of its batch.

Per-core pipeline over 8 row-blocks of 128 queries (software-pipelined:
S for block i+1 is issued on PE before the transposes/attended of block
i, so PE never waits on the exp->mask chain):
    PE   : S-block via fp8 DoubleRow matmuls (xq^T stationary, x^T moving)
    ACT  : exp(scale*S) PSUM->SBUF directly in bf16, free-dim accum -> sa
    DVE  : threshold mask + masked E, all bf16 (2x DVE rate), one reduce
           per half for the kept sum (quantized sum: its rounding error
           cancels against the identically-quantized matmul weights)
    PE   : transpose masked-E chunks (128x128), then attended matmul
           in single-pass bf16 (x as bf16; ~0.2%/elem, rel-err gate 2e-2)
    DVE  : transposed-chunk PSUM->SBUF copies (off ACT so they don't
           queue behind the next block's exp)
    ACT  : scale attended rows by 1/(kept + 1e-9*sa)
    DVE  : add beta * rowsum_W rank-1 term

DMA trigger instructions serialize at ~0.6us each on the issuing queue,
so inputs are staged as FEW large transfers (host pre-lays them out so
each is a contiguous-per-partition 2D pattern), ordered so the transfers
gating the first S matmuls come first.
"""

import sys

sys.path.insert(0, "/opt/trn_rl_repo")

from contextlib import ExitStack

import numpy as np

from concourse import bacc, mybir, tile
from concourse.bass_utils import run_bass_kernel_spmd

F32 = mybir.dt.float32
BF16 = mybir.dt.bfloat16
OP = mybir.AluOpType
AFT = mybir.ActivationFunctionType

P = 128          # partitions / row-block size
T = 2048         # keys per batch
D = 1024         # model dim
QR = 1024        # query rows per core
NBLK = QR // P   # 8 row blocks per core
NT_CHUNKS = T // P    # 16 t-chunks
NC = D // 256    # 4 contraction chunks (fp8 DoubleRow: K=256 each)
NG = T // 512    # 4 column groups for S
NH = 8           # xh transfer groups (2 t-chunks each)
SCALE = float(1.0 / np.sqrt(np.float32(D)))   # 0.03125
THRESH = 0.01
EPS = 1e-9

_CACHE = {}


def _build():
    nc = bacc.Bacc("TRN2", target_bir_lowering=False, debug=False)

    FP8 = mybir.dt.float8e4
    # fp8 DoubleRow operands; layouts put the partition dim second so each
    # [index] slice is one contiguous-per-partition 2D DMA:
    #   xt8[g] = x_b.T, all 4 K-chunks, t-cols 512g..512(g+1)
    #   xq8[i] = x_b.T, all 4 K-chunks, q-cols 128i..128(i+1)
    xt8 = nc.dram_tensor("xt8", [NG, P, NC, 2, 512], FP8, kind="ExternalInput")
    xq8 = nc.dram_tensor("xq8", [NBLK, P, NC, 2, P], FP8, kind="ExternalInput")
    xh = nc.dram_tensor("xh", [NH, P, 2, D], BF16, kind="ExternalInput")
    wb = nc.dram_tensor("wb", [P, D], F32, kind="ExternalInput")       # rowsum_W bcast
    ident = nc.dram_tensor("ident", [P, P], BF16, kind="ExternalInput")
    out = nc.dram_tensor("out", [QR, D], F32, kind="ExternalOutput")

    out_ap = out.ap()

    with tile.TileContext(nc) as tc, ExitStack() as ctx:
        # resident operands
        p_xt = ctx.enter_context(tc.tile_pool(name="xt", bufs=NG))
        p_xq = ctx.enter_context(tc.tile_pool(name="xq", bufs=NBLK))
        p_xn = ctx.enter_context(tc.tile_pool(name="xn", bufs=NH))
        p_cst = ctx.enter_context(tc.tile_pool(name="cst", bufs=3))
        # per-block working tiles
        p_exp = ctx.enter_context(tc.tile_pool(name="exp", bufs=5))
        p_msk = ctx.enter_context(tc.tile_pool(name="msk", bufs=5))
        p_pk = ctx.enter_context(tc.tile_pool(name="pk", bufs=6))
        p_pkt = ctx.enter_context(tc.tile_pool(name="pkt", bufs=7))
        p_out = ctx.enter_context(tc.tile_pool(name="o", bufs=2))
        p_wt = ctx.enter_context(tc.tile_pool(name="wt", bufs=4))
        p_sm = ctx.enter_context(tc.tile_pool(name="sm", bufs=40))
        # PSUM
        p_ps_s = ctx.enter_context(tc.tile_pool(name="ps_s", bufs=2, space="PSUM"))
        p_ps_a = ctx.enter_context(tc.tile_pool(name="ps_a", bufs=2, space="PSUM"))
        p_ps_t = ctx.enter_context(tc.tile_pool(name="ps_t", bufs=2, space="PSUM"))

        # transfers in criticality order; all on the sync queue so earlier
        # ones get the DMA bandwidth first
        xq_sb = [None] * NBLK
        xt_sb = [None] * NG

        def load_xq(i):
            tq = p_xq.tile([P, NC, 2, P], FP8, tag="xq", name="tq")
            nc.sync.dma_start(tq[:], xq8.ap()[i])
            xq_sb[i] = tq

        load_xq(0)
        for g in range(NG):
            tt = p_xt.tile([P, NC, 2, 512], FP8, tag="xt", name="tt")
            nc.sync.dma_start(tt[:], xt8.ap()[g])
            xt_sb[g] = tt
        load_xq(1)
        id_sb = p_cst.tile([P, P], BF16, tag="id")
        nc.sync.dma_start(id_sb[:], ident.ap())
        xh_sb = []
        for jg in range(NH):
            t_ = p_xn.tile([P, 2, D], BF16, tag="xn")
            nc.sync.dma_start(t_[:], xh.ap()[jg])
            xh_sb.append(t_)
            if jg % 2 == 1 and jg // 2 + 2 < NBLK:
                load_xq(jg // 2 + 2)
        for i in range(NH // 2 + 2, NBLK):
            load_xq(i)
        wb_sb = p_cst.tile([P, D], F32, tag="wb")
        nc.sync.dma_start(wb_sb[:], wb.ap())

        def s_block(i):
            """S matmuls + exp for row-block i; returns per-half E and sa."""
            exp_halves, sum_parts = [], []
            for half in range(2):
                sp = p_ps_s.tile([P, T // 2], F32, tag="s", name="sp")
                for k in range(2):
                    g = half * 2 + k
                    for c in range(NC):
                        nc.tensor.matmul(
                            sp[:, k * 512:(k + 1) * 512],
                            lhsT=xq_sb[i][:, c],
                            rhs=xt_sb[g][:, c],
                            start=(c == 0), stop=(c == NC - 1),
                            perf_mode=mybir.MatmulPerfMode.DoubleRow)
                ex = p_exp.tile([P, T // 2], BF16, tag="ex")
                sa = p_sm.tile([P, 1], F32, tag="sm")
                nc.scalar.activation(ex[:], sp[:], AFT.Exp, scale=SCALE,
                                     accum_out=sa[:])
                exp_halves.append(ex)
                sum_parts.append(sa)
            return exp_halves, sum_parts

        def rest_block(i, exp_halves, sum_parts):
            """Mask, transpose, attended matmul, epilogue for row-block i."""
            sum_all = p_sm.tile([P, 1], F32, tag="sm")
            nc.gpsimd.tensor_tensor(sum_all[:], sum_parts[0][:],
                                    sum_parts[1][:], OP.add)
            thr = p_sm.tile([P, 1], F32, tag="sm")
            nc.gpsimd.tensor_scalar(out=thr[:], in0=sum_all[:],
                                    scalar1=THRESH, scalar2=None, op0=OP.mult)

            pk_halves, keptq_parts = [], []
            for half in range(2):
                mk = p_msk.tile([P, T // 2], BF16, tag="mk")
                nc.vector.tensor_scalar(out=mk[:], in0=exp_halves[half][:],
                                        scalar1=thr[:], scalar2=None,
                                        op0=OP.is_ge)
                pk = p_pk.tile([P, T // 2], BF16, tag="pk")
                nc.vector.tensor_tensor(pk[:], exp_halves[half][:], mk[:],
                                        OP.mult)
                pk_halves.append(pk)

            att = [p_ps_a.tile([P, 512], F32, tag="a", name="att")
                   for _ in range(2)]
            # transpose groups into PSUM, one wide copy out per group;
            # first group is a singleton so attended starts sooner
            groups = [[0], [1, 2, 3]] + [
                list(range(g, g + 4)) for g in range(4, NT_CHUNKS, 4)]
            for grp in groups:
                tp = p_ps_t.tile([P, len(grp) * P], BF16, tag="t", name="tp")
                for jj, j in enumerate(grp):
                    src = pk_halves[j // 8][:, (j % 8) * P:(j % 8 + 1) * P]
                    nc.tensor.transpose(tp[:, jj * P:(jj + 1) * P], src,
                                        id_sb[:])
                pkt = p_pkt.tile([P, len(grp) * P], BF16, tag="pkt",
                                 name="pkt")
                nc.vector.tensor_copy(pkt[:], tp[:])
                for jj, j in enumerate(grp):
                    lhs = pkt[:, jj * P:(jj + 1) * P]
                    for k in range(2):
                        dcols = slice(k * 512, (k + 1) * 512)
                        nc.tensor.matmul(att[k][:], lhsT=lhs,
                                         rhs=xh_sb[j // 2][:, j % 2, dcols],
                                         start=(j == 0),
                                         stop=(j == NT_CHUNKS - 1))
            # kept sums off the critical path to attended; the quantized
            # (bf16) sum serves both alpha and beta
            for half in range(2):
                kq = p_sm.tile([P, 1], F32, tag="sm", name="kq")
                nc.vector.tensor_reduce(kq[:], pk_halves[half][:],
                                        mybir.AxisListType.X, OP.add)
                keptq_parts.append(kq)
            kept_q = p_sm.tile([P, 1], F32, tag="sm")
            nc.gpsimd.tensor_tensor(kept_q[:], keptq_parts[0][:],
                                    keptq_parts[1][:], OP.add)

            # alpha = 1 / (kept_q + eps * sum_all)
            den = p_sm.tile([P, 1], F32, tag="sm")
            nc.gpsimd.tensor_scalar(out=den[:], in0=sum_all[:], scalar1=EPS,
                                    scalar2=None, op0=OP.mult)
            nc.gpsimd.tensor_tensor(den[:], den[:], kept_q[:], OP.add)
            alpha = p_sm.tile([P, 1], F32, tag="sm")
            nc.vector.reciprocal(alpha[:], den[:])
            # beta = 0.01 * (1 - kept_q / sum_all)
            rsum = p_sm.tile([P, 1], F32, tag="sm")
            nc.vector.reciprocal(rsum[:], sum_all[:])
            beta = p_sm.tile([P, 1], F32, tag="sm")
            nc.gpsimd.tensor_tensor(beta[:], kept_q[:], rsum[:], OP.mult)
            nc.gpsimd.tensor_scalar(out=beta[:], in0=beta[:], scalar1=-THRESH,
                                    scalar2=THRESH, op0=OP.mult, op1=OP.add)

            # epilogue per d-half so the output DMA overlaps the other half;
            # wt/add on gpsimd so DVE stays free for the next block's
            # mask/mult/copies
            o_sb = p_out.tile([P, D], F32, tag="o")
            for k in range(2):
                dcols = slice(k * 512, (k + 1) * 512)
                nc.scalar.mul(o_sb[:, dcols], att[k][:], alpha[:])
                wt = p_wt.tile([P, 512], F32, tag="wt")
                nc.vector.tensor_scalar(out=wt[:], in0=wb_sb[:, dcols],
                                        scalar1=beta[:], scalar2=None,
                                        op0=OP.mult)
                nc.vector.tensor_tensor(o_sb[:, dcols], o_sb[:, dcols],
                                        wt[:], OP.add)
                nc.sync.dma_start(out_ap[i * P:(i + 1) * P, dcols],
                                  o_sb[:, dcols])

        # software pipeline: S for block i+1 runs on PE while the
        # exp/mask/mult chain of block i fills
        pending = s_block(0)
        for i in range(NBLK):
            nxt = s_block(i + 1) if i + 1 < NBLK else None
            rest_block(i, *pending)
            pending = nxt

    nc.compile()
    return nc


def get_nc():
    if "nc" not in _CACHE:
        _CACHE["nc"] = _build()
    return _CACHE["nc"]


def make_in_maps(x, W):
    import ml_dtypes
    bf = ml_dtypes.bfloat16
    f8 = ml_dtypes.float8_e4m3
    x = np.asarray(x, dtype=np.float32)
    W = np.asarray(W, dtype=np.float32)
    wrow = W.sum(axis=1, dtype=np.float32)                      # (D,)
    wb = np.ascontiguousarray(np.broadcast_to(wrow, (P, D))).astype(np.float32)
    ident = np.eye(P, dtype=bf)
    in_maps = []
    for core in range(8):
        b, h = core // 2, core % 2
        xb = x[b]                                               # (T, D)
        xt_f8 = np.ascontiguousarray(xb.T).astype(f8)           # (D, T)
        xt_dr = xt_f8.reshape(NC, P, 2, T)                      # DoubleRow pairs
        # [g, P, NC, 2, 512]: partition-major contiguous per transfer
        xt8 = np.ascontiguousarray(
            xt_dr.reshape(NC, P, 2, NG, 512).transpose(3, 1, 0, 2, 4))
        xq_dr = xt_dr[:, :, :, h * QR:(h + 1) * QR]             # (NC,P,2,QR)
        xq8 = np.ascontiguousarray(
            xq_dr.reshape(NC, P, 2, NBLK, P).transpose(3, 1, 0, 2, 4))
        xh_bf = np.ascontiguousarray(
            xb.astype(bf).reshape(NH, 2, P, D).transpose(0, 2, 1, 3))
        in_maps.append({"xt8": xt8, "xq8": xq8, "xh": xh_bf,
                        "wb": wb, "ident": ident})
    return in_maps


def kernel(x, W):
    nc = get_nc()
    in_maps = make_in_maps(x, W)
    res = run_bass_kernel_spmd(nc, in_maps, list(range(8)))
    out = np.empty((4, T, D), dtype=np.float32)
    for core in range(8):
        b, h = core // 2, core % 2
        out[b, h * QR:(h + 1) * QR, :] = res.results[core]["out"]
    return out


# revision 18
# speedup vs baseline: 1.0285x; 1.0093x over previous
"""Trainium2 Bass kernel for CasimirSparseAttention.

Math (per batch b):
    S = (x_b @ x_b.T) / sqrt(D)                      # (T, T)
    probs = softmax(S, axis=-1)
    kept = probs >= 0.01  (vacuum = probs < 0.01)
    vac_sum = sum(probs * ~kept)
    casimir[t, o] = vac_sum[t] * rowsum_W[o]          # vac_in is const across D
    attended = (probs*kept) @ x_b / (sum(probs*kept) + 1e-9)
    out = attended + 0.01 * casimir

Working in unnormalized exp-space (E = exp(S/sqrt(D)), row sum = sa):
    kept mask:  E >= 0.01 * sa
    attended = (E*mask) @ x_b / (sum(E*mask) + 1e-9*sa)
    beta     = 0.01 * (1 - sum(E*mask)/sa);  out += beta * rowsum_W

Sharding: 8 cores = (batch b in 0..3) x (half of T). Each core computes
1024 query rows against all 2048 keys of its batch.

Per-core pipeline over 8 row-blocks of 128 queries (software-pipelined:
S for block i+1 is issued on PE before the transposes/attended of block
i, so PE never waits on the exp->mask chain):
    PE   : S-block via fp8 DoubleRow matmuls (xq^T stationary, x^T moving)
    ACT  : exp(scale*S) PSUM->SBUF directly in bf16, free-dim accum -> sa
    DVE  : threshold mask + masked E, all bf16 (2x DVE rate), one reduce
           per half for the kept sum (quantized sum: its rounding error
           cancels against the identically-quantized matmul weights)
    PE   : transpose masked-E chunks (128x128), then attended matmul
           in single-pass bf16 (x as bf16; ~0.2%/elem, rel-err gate 2e-2)
    DVE  : transposed-chunk PSUM->SBUF copies (off ACT so they don't
           queue behind the next block's exp)
    ACT  : scale attended rows by 1/(kept + 1e-9*sa)
    DVE  : add beta * rowsum_W rank-1 term

DMA trigger instructions serialize at ~0.6us each on the issuing queue,
so inputs are staged as FEW large transfers (host pre-lays them out so
each is a contiguous-per-partition 2D pattern), ordered so the transfers
gating the first S matmuls come first.
"""

import sys

sys.path.insert(0, "/opt/trn_rl_repo")

from contextlib import ExitStack

import numpy as np

from concourse import bacc, mybir, tile
from concourse.bass_utils import run_bass_kernel_spmd

F32 = mybir.dt.float32
BF16 = mybir.dt.bfloat16
OP = mybir.AluOpType
AFT = mybir.ActivationFunctionType

P = 128          # partitions / row-block size
T = 2048         # keys per batch
D = 1024         # model dim
QR = 1024        # query rows per core
NBLK = QR // P   # 8 row blocks per core
NT_CHUNKS = T // P    # 16 t-chunks
NC = D // 256    # 4 contraction chunks (fp8 DoubleRow: K=256 each)
NG = T // 512    # 4 column groups for S
NH = 8           # xh transfer groups (2 t-chunks each)
SCALE = float(1.0 / np.sqrt(np.float32(D)))   # 0.03125
THRESH = 0.01
EPS = 1e-9

_CACHE = {}


def _build():
    nc = bacc.Bacc("TRN2", target_bir_lowering=False, debug=False)

    FP8 = mybir.dt.float8e4
    # fp8 DoubleRow operands; layouts put the partition dim second so each
    # [index] slice is one contiguous-per-partition 2D DMA:
    #   xt8[g] = x_b.T, all 4 K-chunks, t-cols 512g..512(g+1)
    #   xq8[i] = x_b.T, all 4 K-chunks, q-cols 128i..128(i+1)
    xt8 = nc.dram_tensor("xt8", [NG, P, NC, 2, 512], FP8, kind="ExternalInput")
    xq8 = nc.dram_tensor("xq8", [NBLK, P, NC, 2, P], FP8, kind="ExternalInput")
    xh = nc.dram_tensor("xh", [NH, P, 2, D], BF16, kind="ExternalInput")
    wb = nc.dram_tensor("wb", [P, D], F32, kind="ExternalInput")       # rowsum_W bcast
    ident = nc.dram_tensor("ident", [P, P], BF16, kind="ExternalInput")
    out = nc.dram_tensor("out", [QR, D], F32, kind="ExternalOutput")

    out_ap = out.ap()

    with tile.TileContext(nc) as tc, ExitStack() as ctx:
        # resident operands
        p_xt = ctx.enter_context(tc.tile_pool(name="xt", bufs=NG))
        p_xq = ctx.enter_context(tc.tile_pool(name="xq", bufs=NBLK))
        p_xn = ctx.enter_context(tc.tile_pool(name="xn", bufs=NH))
        p_cst = ctx.enter_context(tc.tile_pool(name="cst", bufs=3))
        # per-block working tiles
        p_exp = ctx.enter_context(tc.tile_pool(name="exp", bufs=5))
        p_msk = ctx.enter_context(tc.tile_pool(name="msk", bufs=5))
        p_pk = ctx.enter_context(tc.tile_pool(name="pk", bufs=6))
        p_pkt = ctx.enter_context(tc.tile_pool(name="pkt", bufs=7))
        p_out = ctx.enter_context(tc.tile_pool(name="o", bufs=2))
        p_wt = ctx.enter_context(tc.tile_pool(name="wt", bufs=4))
        p_sm = ctx.enter_context(tc.tile_pool(name="sm", bufs=40))
        # PSUM
        p_ps_s = ctx.enter_context(tc.tile_pool(name="ps_s", bufs=2, space="PSUM"))
        p_ps_a = ctx.enter_context(tc.tile_pool(name="ps_a", bufs=2, space="PSUM"))
        p_ps_t = ctx.enter_context(tc.tile_pool(name="ps_t", bufs=2, space="PSUM"))

        # transfers in criticality order; all on the sync queue so earlier
        # ones get the DMA bandwidth first
        xq_sb = [None] * NBLK
        xt_sb = [None] * NG

        def load_xq(i):
            tq = p_xq.tile([P, NC, 2, P], FP8, tag="xq", name="tq")
            nc.sync.dma_start(tq[:], xq8.ap()[i])
            xq_sb[i] = tq

        load_xq(0)
        for g in range(NG):
            tt = p_xt.tile([P, NC, 2, 512], FP8, tag="xt", name="tt")
            nc.sync.dma_start(tt[:], xt8.ap()[g])
            xt_sb[g] = tt
        load_xq(1)
        id_sb = p_cst.tile([P, P], BF16, tag="id")
        nc.sync.dma_start(id_sb[:], ident.ap())
        xh_sb = []
        for jg in range(NH):
            t_ = p_xn.tile([P, 2, D], BF16, tag="xn")
            nc.sync.dma_start(t_[:], xh.ap()[jg])
            xh_sb.append(t_)
            if jg % 2 == 1 and jg // 2 + 2 < NBLK:
                load_xq(jg // 2 + 2)
        for i in range(NH // 2 + 2, NBLK):
            load_xq(i)
        wb_sb = p_cst.tile([P, D], F32, tag="wb")
        nc.sync.dma_start(wb_sb[:], wb.ap())

        def s_block(i):
            """S matmuls + exp for row-block i; returns per-half E and sa."""
            exp_halves, sum_parts = [], []
            for half in range(2):
                sp = p_ps_s.tile([P, T // 2], F32, tag="s", name="sp")
                for k in range(2):
                    g = half * 2 + k
                    for c in range(NC):
                        nc.tensor.matmul(
                            sp[:, k * 512:(k + 1) * 512],
                            lhsT=xq_sb[i][:, c],
                            rhs=xt_sb[g][:, c],
                            start=(c == 0), stop=(c == NC - 1),
                            perf_mode=mybir.MatmulPerfMode.DoubleRow)
                ex = p_exp.tile([P, T // 2], BF16, tag="ex")
                sa = p_sm.tile([P, 1], F32, tag="sm")
                nc.scalar.activation(ex[:], sp[:], AFT.Exp, scale=SCALE,
                                     accum_out=sa[:])
                exp_halves.append(ex)
                sum_parts.append(sa)
            return exp_halves, sum_parts

        def rest_block(i, exp_halves, sum_parts):
            """Mask, transpose, attended matmul, epilogue for row-block i."""
            sum_all = p_sm.tile([P, 1], F32, tag="sm")
            nc.gpsimd.tensor_tensor(sum_all[:], sum_parts[0][:],
                                    sum_parts[1][:], OP.add)
            thr = p_sm.tile([P, 1], F32, tag="sm")
            nc.gpsimd.tensor_scalar(out=thr[:], in0=sum_all[:],
                                    scalar1=THRESH, scalar2=None, op0=OP.mult)

            pk_halves, keptq_parts = [], []
            for half in range(2):
                mk = p_msk.tile([P, T // 2], BF16, tag="mk")
                nc.vector.tensor_scalar(out=mk[:], in0=exp_halves[half][:],
                                        scalar1=thr[:], scalar2=None,
                                        op0=OP.is_ge)
                pk = p_pk.tile([P, T // 2], BF16, tag="pk")
                nc.vector.tensor_tensor(pk[:], exp_halves[half][:], mk[:],
                                        OP.mult)
                pk_halves.append(pk)

            att = [p_ps_a.tile([P, 512], F32, tag="a", name="att")
                   for _ in range(2)]
            # transpose groups into PSUM, one wide copy out per group;
            # first group is a singleton so attended starts sooner
            groups = [[0], [1, 2, 3]] + [
                list(range(g, g + 4)) for g in range(4, NT_CHUNKS, 4)]
            for grp in groups:
                tp = p_ps_t.tile([P, len(grp) * P], BF16, tag="t", name="tp")
                for jj, j in enumerate(grp):
                    src = pk_halves[j // 8][:, (j % 8) * P:(j % 8 + 1) * P]
                    nc.tensor.transpose(tp[:, jj * P:(jj + 1) * P], src,
                                        id_sb[:])
                pkt = p_pkt.tile([P, len(grp) * P], BF16, tag="pkt",
                                 name="pkt")
                nc.vector.tensor_copy(pkt[:], tp[:])
                for jj, j in enumerate(grp):
                    lhs = pkt[:, jj * P:(jj + 1) * P]
                    for k in range(2):
                        dcols = slice(k * 512, (k + 1) * 512)
                        nc.tensor.matmul(att[k][:], lhsT=lhs,
                                         rhs=xh_sb[j // 2][:, j % 2, dcols],
                                         start=(j == 0),
                                         stop=(j == NT_CHUNKS - 1))
            # kept sums off the critical path to attended; the quantized
            # (bf16) sum serves both alpha and beta
            for half in range(2):
                kq = p_sm.tile([P, 1], F32, tag="sm", name="kq")
                nc.vector.tensor_reduce(kq[:], pk_halves[half][:],
                                        mybir.AxisListType.X, OP.add)
                keptq_parts.append(kq)
            kept_q = p_sm.tile([P, 1], F32, tag="sm")
            nc.gpsimd.tensor_tensor(kept_q[:], keptq_parts[0][:],
                                    keptq_parts[1][:], OP.add)

            # alpha = 1 / (kept_q + eps * sum_all)
            den = p_sm.tile([P, 1], F32, tag="sm")
            nc.gpsimd.tensor_scalar(out=den[:], in0=sum_all[:], scalar1=EPS,
                                    scalar2=None, op0=OP.mult)
            nc.gpsimd.tensor_tensor(den[:], den[:], kept_q[:], OP.add)
            alpha = p_sm.tile([P, 1], F32, tag="sm")
            nc.vector.reciprocal(alpha[:], den[:])
            # beta = 0.01 * (1 - kept_q / sum_all)
            rsum = p_sm.tile([P, 1], F32, tag="sm")
            nc.vector.reciprocal(rsum[:], sum_all[:])
            beta = p_sm.tile([P, 1], F32, tag="sm")
            nc.gpsimd.tensor_tensor(beta[:], kept_q[:], rsum[:], OP.mult)
            nc.gpsimd.tensor_scalar(out=beta[:], in0=beta[:], scalar1=-THRESH,
                                    scalar2=THRESH, op0=OP.mult, op1=OP.add)

            # epilogue per d-half so the output DMA overlaps the other half;
            # wt/add on gpsimd so DVE stays free for the next block's
            # mask/mult/copies
            o_sb = p_out.tile([P, D], F32, tag="o")
            for k in range(2):
                dcols = slice(k * 512, (k + 1) * 512)
                nc.scalar.mul(o_sb[:, dcols], att[k][:], alpha[:])
                wt = p_wt.tile([P, 512], F32, tag="wt")
                nc.scalar.mul(wt[:], wb_sb[:, dcols], beta[:])
                nc.gpsimd.tensor_tensor(o_sb[:, dcols], o_sb[:, dcols],
                                        wt[:], OP.add)
                nc.sync.dma_start(out_ap[i * P:(i + 1) * P, dcols],
                                  o_sb[:, dcols])

        # software pipeline: S for block i+1 runs on PE while the
        # exp/mask/mult chain of block i fills
        pending = s_block(0)
        for i in range(NBLK):
            nxt = s_block(i + 1) if i + 1 < NBLK else None
            rest_block(i, *pending)
            pending = nxt

    nc.compile()
    return nc


def get_nc():
    if "nc" not in _CACHE:
        _CACHE["nc"] = _build()
    return _CACHE["nc"]


def make_in_maps(x, W):
    import ml_dtypes
    bf = ml_dtypes.bfloat16
    f8 = ml_dtypes.float8_e4m3
    x = np.asarray(x, dtype=np.float32)
    W = np.asarray(W, dtype=np.float32)
    wrow = W.sum(axis=1, dtype=np.float32)                      # (D,)
    wb = np.ascontiguousarray(np.broadcast_to(wrow, (P, D))).astype(np.float32)
    ident = np.eye(P, dtype=bf)
    in_maps = []
    for core in range(8):
        b, h = core // 2, core % 2
        xb = x[b]                                               # (T, D)
        xt_f8 = np.ascontiguousarray(xb.T).astype(f8)           # (D, T)
        xt_dr = xt_f8.reshape(NC, P, 2, T)                      # DoubleRow pairs
        # [g, P, NC, 2, 512]: partition-major contiguous per transfer
        xt8 = np.ascontiguousarray(
            xt_dr.reshape(NC, P, 2, NG, 512).transpose(3, 1, 0, 2, 4))
        xq_dr = xt_dr[:, :, :, h * QR:(h + 1) * QR]             # (NC,P,2,QR)
        xq8 = np.ascontiguousarray(
            xq_dr.reshape(NC, P, 2, NBLK, P).transpose(3, 1, 0, 2, 4))
        xh_bf = np.ascontiguousarray(
            xb.astype(bf).reshape(NH, 2, P, D).transpose(0, 2, 1, 3))
        in_maps.append({"xt8": xt8, "xq8": xq8, "xh": xh_bf,
                        "wb": wb, "ident": ident})
    return in_maps


def kernel(x, W):
    nc = get_nc()
    in_maps = make_in_maps(x, W)
    res = run_bass_kernel_spmd(nc, in_maps, list(range(8)))
    out = np.empty((4, T, D), dtype=np.float32)
    for core in range(8):
        b, h = core // 2, core % 2
        out[b, h * QR:(h + 1) * QR, :] = res.results[core]["out"]
    return out
